# revision 2
# baseline (speedup 1.0000x reference)
"""2-layer GCN (GCNConv -> ReLU -> GCNConv) on 8 TRN2 NeuronCores.

Strategy (sliced-ELLPACK, node sharding):
  GCN algebra: out = D^-1/2 (A+I) D^-1/2 (relu(D^-1/2 (A+I) D^-1/2 x W1 + b1)) W2 + b2.
  The normalization is separable (norm_e = dinv[row]*dinv[col]) and aggregation
  is linear, so each layer is: per-node scale -> unweighted neighbor-sum ->
  per-node scale -> dense matmul. Self-loop contributions are dense adds.

  Host (pure index layout, no model math): sort nodes by in-degree, pack each
  node's incoming edges into a padded row of slots (sliced ELLPACK, slices of
  128 nodes, groups of 32 slices sharing a width). Per-slot it gathers the
  *input* features x[row] and the integer structural count deg[row]. Device
  computes all floating point model math: rsqrt, scalings, segmented
  reductions (DVE tensor_reduce), both layer matmuls, relu.

  Two launches: A computes layer 1 + the per-node scalar sigma = (dinv*h)@W2;
  host re-shards sigma into the same slot layout (index gather only);
  B reduces sigma-slots and finishes layer 2. Nodes are sharded across the 8
  cores round-robin by 128-node slice; edge slots live with their target node.
"""
import numpy as np

P = 128
N_CORES = 8
NSL = 256           # local slices per core  -> 2048 global slices
NGRP = 8            # groups of 32 slices sharing one slot width
NRANKS = 2048 * P   # padded rank space (262144 >= 250000)

TRACE = False
_cache = {}


def _install_ntff_shim():
    import contextlib, ctypes, sys, types
    if "antenv.axon_hooks" in sys.modules:
        return
    try:
        lib = ctypes.CDLL("/opt/axon/libaxon_pjrt.so")
        if not hasattr(lib, "axon_start_nrt_profile"):
            return
        lib.axon_start_nrt_profile.argtypes = [ctypes.POINTER(ctypes.c_int64), ctypes.c_size_t]
        lib.axon_start_nrt_profile.restype = ctypes.c_int64
        lib.axon_stop_nrt_profile.argtypes = [ctypes.c_char_p]
        lib.axon_stop_nrt_profile.restype = ctypes.c_int64
    except OSError:
        return

    @contextlib.contextmanager
    def _hook(output_dir, device_ids):
        import jax
        jax.devices()
        if device_ids:
            ids = (ctypes.c_int64 * len(device_ids))(*device_ids)
            rc = lib.axon_start_nrt_profile(ids, len(device_ids))
        else:
            rc = lib.axon_start_nrt_profile(None, 0)
        if rc != 0:
            raise RuntimeError(f"axon_start_nrt_profile rc={rc}")
        try:
            yield
        finally:
            lib.axon_stop_nrt_profile(str(output_dir).encode())

    mod = types.ModuleType("antenv.axon_hooks")
    mod.get_axon_ntff_profile_hook = lambda: _hook
    mod.set_axon_ntff_profile_hook = lambda h: None
    sys.modules["antenv.axon_hooks"] = mod


def _build_programs(TOT, W_grp, off):
    import concourse.bass as bass
    import concourse.bacc as bacc
    import concourse.tile as tile
    import concourse.mybir as mybir

    f32 = mybir.dt.float32
    AF = mybir.ActivationFunctionType
    ALU = mybir.AluOpType

    def reduce_groups(nc, tc, pool, slots_dram, agg_tile):
        """agg_tile[:, g*32:(g+1)*32] = segmented sums of the slot array."""
        for g in range(NGRP):
            Wg = int(W_grp[g])
            if Wg == 0:
                continue
            t = pool.tile([P, 32 * Wg], f32, tag="slots")
            nc.sync.dma_start(t[:], slots_dram.ap()[:, int(off[g]):int(off[g]) + 32 * Wg])
            yield g, Wg, t

    # ---------------- program A ----------------
    ncA = bacc.Bacc("TRN2", target_bir_lowering=False, debug=False, num_devices=N_CORES)
    x0s = ncA.dram_tensor("x0s", [P, TOT], f32, kind="ExternalInput")
    x1s = ncA.dram_tensor("x1s", [P, TOT], f32, kind="ExternalInput")
    dgs = ncA.dram_tensor("dgs", [P, TOT], f32, kind="ExternalInput")
    x0t = ncA.dram_tensor("x0t", [P, NSL], f32, kind="ExternalInput")
    x1t = ncA.dram_tensor("x1t", [P, NSL], f32, kind="ExternalInput")
    dgt = ncA.dram_tensor("dgt", [P, NSL], f32, kind="ExternalInput")
    wb = ncA.dram_tensor("wb", [P, 65], f32, kind="ExternalInput")
    sOut = ncA.dram_tensor("sOut", [P, NSL], f32, kind="ExternalOutput")

    with tile.TileContext(ncA) as tc:
        with tc.tile_pool(name="slots", bufs=3) as pool, \
             tc.tile_pool(name="persist", bufs=1) as pp:
            agg0 = pp.tile([P, NSL], f32)
            agg1 = pp.tile([P, NSL], f32)
            for g in range(NGRP):
                Wg = int(W_grp[g])
                o = int(off[g])
                if Wg == 0:
                    ncA.gpsimd.memset(agg0[:, g * 32:(g + 1) * 32], 0.0)
                    ncA.gpsimd.memset(agg1[:, g * 32:(g + 1) * 32], 0.0)
                    continue
                td = pool.tile([P, 32 * Wg], f32, tag="dg")
                ncA.sync.dma_start(td[:], dgs.ap()[:, o:o + 32 * Wg])
                t0 = pool.tile([P, 32 * Wg], f32, tag="x0")
                ncA.sync.dma_start(t0[:], x0s.ap()[:, o:o + 32 * Wg])
                t1 = pool.tile([P, 32 * Wg], f32, tag="x1")
                ncA.sync.dma_start(t1[:], x1s.ap()[:, o:o + 32 * Wg])
                r = pool.tile([P, 32 * Wg], f32, tag="r")
                ncA.scalar.activation(r[:], td[:], AF.Sqrt)
                ncA.vector.reciprocal(r[:], r[:])
                ncA.vector.tensor_tensor(out=t0[:], in0=t0[:], in1=r[:], op=ALU.mult)
                ncA.vector.tensor_tensor(out=t1[:], in0=t1[:], in1=r[:], op=ALU.mult)
                ncA.vector.tensor_reduce(
                    out=agg0[:, g * 32:(g + 1) * 32],
                    in_=t0[:].rearrange("p (n w) -> p n w", w=Wg),
                    axis=mybir.AxisListType.X, op=ALU.add)
                ncA.vector.tensor_reduce(
                    out=agg1[:, g * 32:(g + 1) * 32],
                    in_=t1[:].rearrange("p (n w) -> p n w", w=Wg),
                    axis=mybir.AxisListType.X, op=ALU.add)

            # dense per-node chain
            xt0 = pp.tile([P, NSL], f32)
            ncA.sync.dma_start(xt0[:], x0t.ap())
            xt1 = pp.tile([P, NSL], f32)
            ncA.sync.dma_start(xt1[:], x1t.ap())
            dt = pp.tile([P, NSL], f32)
            ncA.sync.dma_start(dt[:], dgt.ap())
            wbt = pp.tile([P, 65], f32)
            ncA.sync.dma_start(wbt[:], wb.ap())

            dinv = pp.tile([P, NSL], f32)
            ncA.scalar.activation(dinv[:], dt[:], AF.Sqrt)
            ncA.vector.reciprocal(dinv[:], dinv[:])
            # z_f = dinv * (agg_f + dinv * x_f)
            tmp = pp.tile([P, NSL], f32)
            z0 = pp.tile([P, NSL], f32)
            z1 = pp.tile([P, NSL], f32)
            ncA.vector.tensor_tensor(out=tmp[:], in0=dinv[:], in1=xt0[:], op=ALU.mult)
            ncA.vector.tensor_tensor(out=tmp[:], in0=tmp[:], in1=agg0[:], op=ALU.add)
            ncA.vector.tensor_tensor(out=z0[:], in0=tmp[:], in1=dinv[:], op=ALU.mult)
            ncA.vector.tensor_tensor(out=tmp[:], in0=dinv[:], in1=xt1[:], op=ALU.mult)
            ncA.vector.tensor_tensor(out=tmp[:], in0=tmp[:], in1=agg1[:], op=ALU.add)
            ncA.vector.tensor_tensor(out=z1[:], in0=tmp[:], in1=dinv[:], op=ALU.mult)

            # h_j = relu(z0*W1[0,j] + z1*W1[1,j] + b1[j]); acc = sum_j h_j*W2[j]
            acc = pp.tile([P, NSL], f32)
            hj = pp.tile([P, NSL], f32)
            tmp2 = pp.tile([P, NSL], f32)
            for j in range(16):
                ncA.vector.tensor_tensor(
                    out=hj[:], in0=z0[:],
                    in1=wbt[:, j:j + 1].to_broadcast([P, NSL]), op=ALU.mult)
                ncA.vector.tensor_tensor(
                    out=tmp2[:], in0=z1[:],
                    in1=wbt[:, 16 + j:17 + j].to_broadcast([P, NSL]), op=ALU.mult)
                ncA.vector.tensor_tensor(out=hj[:], in0=hj[:], in1=tmp2[:], op=ALU.add)
                ncA.vector.tensor_tensor(
                    out=hj[:], in0=hj[:],
                    in1=wbt[:, 32 + j:33 + j].to_broadcast([P, NSL]), op=ALU.add)
                ncA.scalar.activation(hj[:], hj[:], AF.Relu)
                ncA.vector.tensor_tensor(
                    out=hj[:], in0=hj[:],
                    in1=wbt[:, 48 + j:49 + j].to_broadcast([P, NSL]), op=ALU.mult)
                if j == 0:
                    ncA.vector.tensor_copy(acc[:], hj[:])
                else:
                    ncA.vector.tensor_tensor(out=acc[:], in0=acc[:], in1=hj[:], op=ALU.add)
            # sigma = dinv * acc
            ncA.vector.tensor_tensor(out=acc[:], in0=acc[:], in1=dinv[:], op=ALU.mult)
            ncA.sync.dma_start(sOut.ap(), acc[:])
    ncA.compile()

    # ---------------- program B ----------------
    ncB = bacc.Bacc("TRN2", target_bir_lowering=False, debug=False, num_devices=N_CORES)
    sgs = ncB.dram_tensor("sgs", [P, TOT], f32, kind="ExternalInput")
    dgtB = ncB.dram_tensor("dgtB", [P, NSL], f32, kind="ExternalInput")
    stB = ncB.dram_tensor("stB", [P, NSL], f32, kind="ExternalInput")
    wbB = ncB.dram_tensor("wbB", [P, 65], f32, kind="ExternalInput")
    out = ncB.dram_tensor("out", [P, NSL], f32, kind="ExternalOutput")

    with tile.TileContext(ncB) as tc:
        with tc.tile_pool(name="slots", bufs=3) as pool, \
             tc.tile_pool(name="persist", bufs=1) as pp:
            agg = pp.tile([P, NSL], f32)
            for g in range(NGRP):
                Wg = int(W_grp[g])
                o = int(off[g])
                if Wg == 0:
                    ncB.gpsimd.memset(agg[:, g * 32:(g + 1) * 32], 0.0)
                    continue
                t = pool.tile([P, 32 * Wg], f32, tag="s")
                ncB.sync.dma_start(t[:], sgs.ap()[:, o:o + 32 * Wg])
                ncB.vector.tensor_reduce(
                    out=agg[:, g * 32:(g + 1) * 32],
                    in_=t[:].rearrange("p (n w) -> p n w", w=Wg),
                    axis=mybir.AxisListType.X, op=ALU.add)
            dt = pp.tile([P, NSL], f32)
            ncB.sync.dma_start(dt[:], dgtB.ap())
            st = pp.tile([P, NSL], f32)
            ncB.sync.dma_start(st[:], stB.ap())
            wbt = pp.tile([P, 65], f32)
            ncB.sync.dma_start(wbt[:], wbB.ap())
            dinv = pp.tile([P, NSL], f32)
            ncB.scalar.activation(dinv[:], dt[:], AF.Sqrt)
            ncB.vector.reciprocal(dinv[:], dinv[:])
            o1 = pp.tile([P, NSL], f32)
            ncB.vector.tensor_tensor(out=o1[:], in0=agg[:], in1=st[:], op=ALU.add)
            ncB.vector.tensor_tensor(out=o1[:], in0=o1[:], in1=dinv[:], op=ALU.mult)
            ncB.vector.tensor_tensor(
                out=o1[:], in0=o1[:],
                in1=wbt[:, 64:65].to_broadcast([P, NSL]), op=ALU.add)
            ncB.sync.dma_start(out.ap(), o1[:])
    ncB.compile()
    return ncA, ncB


def kernel(x, edge_index, W1, b1, W2, b2, n_nodes):
    from concourse.bass_utils import run_bass_kernel_spmd

    N = int(n_nodes)
    x = np.asarray(x, dtype=np.float32)
    ei = np.asarray(edge_index)
    row = ei[0].astype(np.int64)
    col = ei[1].astype(np.int64)
    W1 = np.asarray(W1, np.float32); b1 = np.asarray(b1, np.float32)
    W2 = np.asarray(W2, np.float32); b2 = np.asarray(b2, np.float32)
    E = row.shape[0]

    # ---- host index layout (structural only) ----
    deg = np.bincount(col, minlength=N) + 1           # includes self-loop
    indeg = deg - 1
    order = np.argsort(-deg, kind="stable")           # rank -> node
    rank_of = np.empty(N, np.int64)
    rank_of[order] = np.arange(N)

    indeg_byrank = np.zeros(NRANKS, np.int64)
    indeg_byrank[:N] = indeg[order]
    W_slice = indeg_byrank.reshape(2048, P).max(axis=1)          # per global slice
    W_by_l = W_slice.reshape(NSL, N_CORES).max(axis=1)           # max over cores
    W_grp = W_by_l.reshape(NGRP, 32).max(axis=1)                 # per group
    off = np.zeros(NGRP, np.int64)
    np.cumsum(32 * W_grp[:-1], out=off[1:])
    TOT = int(off[-1] + 32 * W_grp[-1])

    key = (TOT, tuple(W_grp.tolist()))
    if key not in _cache:
        if TRACE:
            _install_ntff_shim()
        _cache[key] = _build_programs(TOT, W_grp, off)
    ncA, ncB = _cache[key]

    # ---- per-edge slot placement ----
    re = rank_of[col]
    sl = re >> 7
    pe = re & 127
    ce = sl % N_CORES
    le = sl // N_CORES
    ge = le >> 5
    sidx = np.argsort(re, kind="stable")
    re_s = re[sidx]
    runstart = np.empty(E, bool)
    runstart[0] = True
    np.not_equal(re_s[1:], re_s[:-1], out=runstart[1:])
    starts = np.flatnonzero(runstart)
    rid = np.cumsum(runstart) - 1
    slot = np.empty(E, np.int64)
    slot[sidx] = np.arange(E) - starts[rid]
    pos = off[ge] + (le - (ge << 5)) * W_grp[ge] + slot

    x0s = np.zeros((N_CORES, P, TOT), np.float32)
    x1s = np.zeros((N_CORES, P, TOT), np.float32)
    dgs = np.ones((N_CORES, P, TOT), np.float32)
    core_masks = []
    for c in range(N_CORES):
        m = ce == c
        core_masks.append(m)
        x0s[c][pe[m], pos[m]] = x[row[m], 0]
        x1s[c][pe[m], pos[m]] = x[row[m], 1]
        dgs[c][pe[m], pos[m]] = deg[row[m]]

    # ---- node tables ----
    pgrid = np.arange(P)[:, None]
    lgrid = np.arange(NSL)[None, :]
    x0t = np.zeros((N_CORES, P, NSL), np.float32)
    x1t = np.zeros((N_CORES, P, NSL), np.float32)
    dgt = np.ones((N_CORES, P, NSL), np.float32)
    nodes_c = []
    valid_c = []
    for c in range(N_CORES):
        ranks = (lgrid * N_CORES + c) * P + pgrid          # [P, NSL]
        valid = ranks < N
        nodes = order[np.minimum(ranks, N - 1)]
        nodes_c.append(nodes); valid_c.append(valid)
        x0t[c] = np.where(valid, x[nodes, 0], 0.0)
        x1t[c] = np.where(valid, x[nodes, 1], 0.0)
        dgt[c] = np.where(valid, deg[nodes].astype(np.float32), 1.0)

    wb = np.zeros((P, 65), np.float32)
    wb[:, 0:16] = W1[0]; wb[:, 16:32] = W1[1]
    wb[:, 32:48] = b1
    wb[:, 48:64] = W2[:, 0]
    wb[:, 64] = b2[0]

    in_maps_A = [{"x0s": x0s[c], "x1s": x1s[c], "dgs": dgs[c],
                  "x0t": x0t[c], "x1t": x1t[c], "dgt": dgt[c], "wb": wb}
                 for c in range(N_CORES)]
    resA = run_bass_kernel_spmd(ncA, in_maps_A, core_ids=list(range(N_CORES)),
                                trace=TRACE)

    # ---- sigma table, host re-shard into slots ----
    sigma = np.zeros(N, np.float32)
    for c in range(N_CORES):
        v = valid_c[c]
        sigma[nodes_c[c][v]] = resA.results[c]["sOut"][v]

    sgs = np.zeros((N_CORES, P, TOT), np.float32)
    st = np.zeros((N_CORES, P, NSL), np.float32)
    for c in range(N_CORES):
        m = core_masks[c]
        sgs[c][pe[m], pos[m]] = sigma[row[m]]
        st[c] = np.where(valid_c[c], sigma[nodes_c[c]], 0.0)

    in_maps_B = [{"sgs": sgs[c], "dgtB": dgt[c], "stB": st[c], "wbB": wb}
                 for c in range(N_CORES)]
    resB = run_bass_kernel_spmd(ncB, in_maps_B, core_ids=list(range(N_CORES)),
                                trace=TRACE)

    out = np.zeros(N, np.float32)
    for c in range(N_CORES):
        v = valid_c[c]
        out[nodes_c[c][v]] = resB.results[c]["out"][v]

    kernel.last_exec_ns = (getattr(resA, "exec_time_ns", None) or 0) + \
                          (getattr(resB, "exec_time_ns", None) or 0)
    return out[:, None]


# revision 3
# speedup vs baseline: 1.0594x; 1.0594x over previous
"""2-layer GCN (GCNConv -> ReLU -> GCNConv) on 8 TRN2 NeuronCores.

Strategy (sliced-ELLPACK, node sharding):
  GCN algebra: out = D^-1/2 (A+I) D^-1/2 (relu(D^-1/2 (A+I) D^-1/2 x W1 + b1)) W2 + b2.
  The normalization is separable (norm_e = dinv[row]*dinv[col]) and aggregation
  is linear, so each layer is: per-node scale -> unweighted neighbor-sum ->
  per-node scale -> dense matmul. Self-loop contributions are dense adds.

  Host (pure index layout, no model math): sort nodes by in-degree, pack each
  node's incoming edges into a padded row of slots (sliced ELLPACK, slices of
  128 nodes, groups of 32 slices sharing a width). Per-slot it gathers the
  *input* features x[row] and the integer structural count deg[row]. Device
  computes all floating point model math: rsqrt, scalings, segmented
  reductions (DVE tensor_reduce), both layer matmuls, relu.

  Two launches: A computes layer 1 + the per-node scalar sigma = (dinv*h)@W2;
  host re-shards sigma into the same slot layout (index gather only);
  B reduces sigma-slots and finishes layer 2. Nodes are sharded across the 8
  cores round-robin by 128-node slice; edge slots live with their target node.
"""
import numpy as np

P = 128
N_CORES = 8
NSL = 256           # local slices per core  -> 2048 global slices
NGRP = 8            # groups of 32 slices sharing one slot width
NRANKS = 2048 * P   # padded rank space (262144 >= 250000)

TRACE = False
_cache = {}


def _install_ntff_shim():
    import contextlib, ctypes, sys, types
    if "antenv.axon_hooks" in sys.modules:
        return
    try:
        lib = ctypes.CDLL("/opt/axon/libaxon_pjrt.so")
        if not hasattr(lib, "axon_start_nrt_profile"):
            return
        lib.axon_start_nrt_profile.argtypes = [ctypes.POINTER(ctypes.c_int64), ctypes.c_size_t]
        lib.axon_start_nrt_profile.restype = ctypes.c_int64
        lib.axon_stop_nrt_profile.argtypes = [ctypes.c_char_p]
        lib.axon_stop_nrt_profile.restype = ctypes.c_int64
    except OSError:
        return

    @contextlib.contextmanager
    def _hook(output_dir, device_ids):
        import jax
        jax.devices()
        if device_ids:
            ids = (ctypes.c_int64 * len(device_ids))(*device_ids)
            rc = lib.axon_start_nrt_profile(ids, len(device_ids))
        else:
            rc = lib.axon_start_nrt_profile(None, 0)
        if rc != 0:
            raise RuntimeError(f"axon_start_nrt_profile rc={rc}")
        try:
            yield
        finally:
            lib.axon_stop_nrt_profile(str(output_dir).encode())

    mod = types.ModuleType("antenv.axon_hooks")
    mod.get_axon_ntff_profile_hook = lambda: _hook
    mod.set_axon_ntff_profile_hook = lambda h: None
    sys.modules["antenv.axon_hooks"] = mod


def _build_programs(TOT, W_grp, off):
    import concourse.bass as bass
    import concourse.bacc as bacc
    import concourse.tile as tile
    import concourse.mybir as mybir

    f32 = mybir.dt.float32
    AF = mybir.ActivationFunctionType
    ALU = mybir.AluOpType

    def reduce_groups(nc, tc, pool, slots_dram, agg_tile):
        """agg_tile[:, g*32:(g+1)*32] = segmented sums of the slot array."""
        for g in range(NGRP):
            Wg = int(W_grp[g])
            if Wg == 0:
                continue
            t = pool.tile([P, 32 * Wg], f32, tag="slots")
            nc.sync.dma_start(t[:], slots_dram.ap()[:, int(off[g]):int(off[g]) + 32 * Wg])
            yield g, Wg, t

    # ---------------- program A ----------------
    ncA = bacc.Bacc("TRN2", target_bir_lowering=False, debug=False, num_devices=N_CORES)
    x0s = ncA.dram_tensor("x0s", [P, TOT], f32, kind="ExternalInput")
    x1s = ncA.dram_tensor("x1s", [P, TOT], f32, kind="ExternalInput")
    dgs = ncA.dram_tensor("dgs", [P, TOT], f32, kind="ExternalInput")
    x0t = ncA.dram_tensor("x0t", [P, NSL], f32, kind="ExternalInput")
    x1t = ncA.dram_tensor("x1t", [P, NSL], f32, kind="ExternalInput")
    dgt = ncA.dram_tensor("dgt", [P, NSL], f32, kind="ExternalInput")
    wb = ncA.dram_tensor("wb", [P, 65], f32, kind="ExternalInput")
    sOut = ncA.dram_tensor("sOut", [P, NSL], f32, kind="ExternalOutput")

    with tile.TileContext(ncA) as tc:
        with tc.tile_pool(name="slots", bufs=4) as pool, \
             tc.tile_pool(name="persist", bufs=1) as pp:
            agg0 = pp.tile([P, NSL], f32)
            agg1 = pp.tile([P, NSL], f32)
            xt0 = pp.tile([P, NSL], f32)
            ncA.sync.dma_start(xt0[:], x0t.ap())
            xt1 = pp.tile([P, NSL], f32)
            ncA.sync.dma_start(xt1[:], x1t.ap())
            dt = pp.tile([P, NSL], f32)
            ncA.sync.dma_start(dt[:], dgt.ap())
            wbt = pp.tile([P, 65], f32)
            ncA.sync.dma_start(wbt[:], wb.ap())
            dinv = pp.tile([P, NSL], f32)
            ncA.scalar.activation(dinv[:], dt[:], AF.Sqrt)
            ncA.vector.reciprocal(dinv[:], dinv[:])
            for g in range(NGRP):
                Wg = int(W_grp[g])
                o = int(off[g])
                if Wg == 0:
                    ncA.gpsimd.memset(agg0[:, g * 32:(g + 1) * 32], 0.0)
                    ncA.gpsimd.memset(agg1[:, g * 32:(g + 1) * 32], 0.0)
                    continue
                td = pool.tile([P, 32 * Wg], f32, tag="dg")
                ncA.sync.dma_start(td[:], dgs.ap()[:, o:o + 32 * Wg])
                t0 = pool.tile([P, 32 * Wg], f32, tag="x0")
                ncA.sync.dma_start(t0[:], x0s.ap()[:, o:o + 32 * Wg])
                t1 = pool.tile([P, 32 * Wg], f32, tag="x1")
                ncA.sync.dma_start(t1[:], x1s.ap()[:, o:o + 32 * Wg])
                r = pool.tile([P, 32 * Wg], f32, tag="r")
                ncA.scalar.activation(r[:], td[:], AF.Sqrt)
                ncA.vector.reciprocal(r[:], r[:])
                ncA.vector.tensor_tensor(out=t0[:], in0=t0[:], in1=r[:], op=ALU.mult)
                ncA.vector.tensor_tensor(out=t1[:], in0=t1[:], in1=r[:], op=ALU.mult)
                ncA.vector.tensor_reduce(
                    out=agg0[:, g * 32:(g + 1) * 32],
                    in_=t0[:].rearrange("p (n w) -> p n w", w=Wg),
                    axis=mybir.AxisListType.X, op=ALU.add)
                ncA.vector.tensor_reduce(
                    out=agg1[:, g * 32:(g + 1) * 32],
                    in_=t1[:].rearrange("p (n w) -> p n w", w=Wg),
                    axis=mybir.AxisListType.X, op=ALU.add)

            # z_f = dinv * (agg_f + dinv * x_f)
            tmp = pp.tile([P, NSL], f32)
            z0 = pp.tile([P, NSL], f32)
            z1 = pp.tile([P, NSL], f32)
            ncA.vector.tensor_tensor(out=tmp[:], in0=dinv[:], in1=xt0[:], op=ALU.mult)
            ncA.vector.tensor_tensor(out=tmp[:], in0=tmp[:], in1=agg0[:], op=ALU.add)
            ncA.vector.tensor_tensor(out=z0[:], in0=tmp[:], in1=dinv[:], op=ALU.mult)
            ncA.vector.tensor_tensor(out=tmp[:], in0=dinv[:], in1=xt1[:], op=ALU.mult)
            ncA.vector.tensor_tensor(out=tmp[:], in0=tmp[:], in1=agg1[:], op=ALU.add)
            ncA.vector.tensor_tensor(out=z1[:], in0=tmp[:], in1=dinv[:], op=ALU.mult)

            # h_j = relu(z0*W1[0,j] + z1*W1[1,j] + b1[j]); acc = sum_j h_j*W2[j]
            acc = pp.tile([P, NSL], f32)
            hj = pp.tile([P, NSL], f32)
            tmp2 = pp.tile([P, NSL], f32)
            for j in range(16):
                ncA.vector.tensor_tensor(
                    out=hj[:], in0=z0[:],
                    in1=wbt[:, j:j + 1].to_broadcast([P, NSL]), op=ALU.mult)
                ncA.vector.tensor_tensor(
                    out=tmp2[:], in0=z1[:],
                    in1=wbt[:, 16 + j:17 + j].to_broadcast([P, NSL]), op=ALU.mult)
                ncA.vector.tensor_tensor(out=hj[:], in0=hj[:], in1=tmp2[:], op=ALU.add)
                ncA.scalar.activation(hj[:], hj[:], AF.Relu,
                                      bias=wbt[:, 32 + j:33 + j])
                ncA.vector.tensor_tensor(
                    out=hj[:], in0=hj[:],
                    in1=wbt[:, 48 + j:49 + j].to_broadcast([P, NSL]), op=ALU.mult)
                if j == 0:
                    ncA.vector.tensor_copy(acc[:], hj[:])
                else:
                    ncA.vector.tensor_tensor(out=acc[:], in0=acc[:], in1=hj[:], op=ALU.add)
            # sigma = dinv * acc
            ncA.vector.tensor_tensor(out=acc[:], in0=acc[:], in1=dinv[:], op=ALU.mult)
            ncA.sync.dma_start(sOut.ap(), acc[:])
    ncA.compile()

    # ---------------- program B ----------------
    ncB = bacc.Bacc("TRN2", target_bir_lowering=False, debug=False, num_devices=N_CORES)
    sgs = ncB.dram_tensor("sgs", [P, TOT], f32, kind="ExternalInput")
    dgtB = ncB.dram_tensor("dgtB", [P, NSL], f32, kind="ExternalInput")
    stB = ncB.dram_tensor("stB", [P, NSL], f32, kind="ExternalInput")
    wbB = ncB.dram_tensor("wbB", [P, 65], f32, kind="ExternalInput")
    out = ncB.dram_tensor("out", [P, NSL], f32, kind="ExternalOutput")

    with tile.TileContext(ncB) as tc:
        with tc.tile_pool(name="slots", bufs=3) as pool, \
             tc.tile_pool(name="persist", bufs=1) as pp:
            agg = pp.tile([P, NSL], f32)
            for g in range(NGRP):
                Wg = int(W_grp[g])
                o = int(off[g])
                if Wg == 0:
                    ncB.gpsimd.memset(agg[:, g * 32:(g + 1) * 32], 0.0)
                    continue
                t = pool.tile([P, 32 * Wg], f32, tag="s")
                ncB.sync.dma_start(t[:], sgs.ap()[:, o:o + 32 * Wg])
                ncB.vector.tensor_reduce(
                    out=agg[:, g * 32:(g + 1) * 32],
                    in_=t[:].rearrange("p (n w) -> p n w", w=Wg),
                    axis=mybir.AxisListType.X, op=ALU.add)
            dt = pp.tile([P, NSL], f32)
            ncB.sync.dma_start(dt[:], dgtB.ap())
            st = pp.tile([P, NSL], f32)
            ncB.sync.dma_start(st[:], stB.ap())
            wbt = pp.tile([P, 65], f32)
            ncB.sync.dma_start(wbt[:], wbB.ap())
            dinv = pp.tile([P, NSL], f32)
            ncB.scalar.activation(dinv[:], dt[:], AF.Sqrt)
            ncB.vector.reciprocal(dinv[:], dinv[:])
            o1 = pp.tile([P, NSL], f32)
            ncB.vector.tensor_tensor(out=o1[:], in0=agg[:], in1=st[:], op=ALU.add)
            ncB.vector.tensor_tensor(out=o1[:], in0=o1[:], in1=dinv[:], op=ALU.mult)
            ncB.vector.tensor_tensor(
                out=o1[:], in0=o1[:],
                in1=wbt[:, 64:65].to_broadcast([P, NSL]), op=ALU.add)
            ncB.sync.dma_start(out.ap(), o1[:])
    ncB.compile()
    return ncA, ncB


def kernel(x, edge_index, W1, b1, W2, b2, n_nodes):
    from concourse.bass_utils import run_bass_kernel_spmd

    N = int(n_nodes)
    x = np.asarray(x, dtype=np.float32)
    ei = np.asarray(edge_index)
    row = ei[0].astype(np.int64)
    col = ei[1].astype(np.int64)
    W1 = np.asarray(W1, np.float32); b1 = np.asarray(b1, np.float32)
    W2 = np.asarray(W2, np.float32); b2 = np.asarray(b2, np.float32)
    E = row.shape[0]

    # ---- host index layout (structural only) ----
    deg = np.bincount(col, minlength=N) + 1           # includes self-loop
    indeg = deg - 1
    order = np.argsort(-deg, kind="stable")           # rank -> node
    rank_of = np.empty(N, np.int64)
    rank_of[order] = np.arange(N)

    indeg_byrank = np.zeros(NRANKS, np.int64)
    indeg_byrank[:N] = indeg[order]
    W_slice = indeg_byrank.reshape(2048, P).max(axis=1)          # per global slice
    W_by_l = W_slice.reshape(NSL, N_CORES).max(axis=1)           # max over cores
    W_grp = W_by_l.reshape(NGRP, 32).max(axis=1)                 # per group
    off = np.zeros(NGRP, np.int64)
    np.cumsum(32 * W_grp[:-1], out=off[1:])
    TOT = int(off[-1] + 32 * W_grp[-1])

    key = (TOT, tuple(W_grp.tolist()))
    if key not in _cache:
        if TRACE:
            _install_ntff_shim()
        _cache[key] = _build_programs(TOT, W_grp, off)
    ncA, ncB = _cache[key]

    # ---- per-edge slot placement ----
    re = rank_of[col]
    sl = re >> 7
    pe = re & 127
    ce = sl % N_CORES
    le = sl // N_CORES
    ge = le >> 5
    sidx = np.argsort(re, kind="stable")
    re_s = re[sidx]
    runstart = np.empty(E, bool)
    runstart[0] = True
    np.not_equal(re_s[1:], re_s[:-1], out=runstart[1:])
    starts = np.flatnonzero(runstart)
    rid = np.cumsum(runstart) - 1
    slot = np.empty(E, np.int64)
    slot[sidx] = np.arange(E) - starts[rid]
    pos = off[ge] + (le - (ge << 5)) * W_grp[ge] + slot

    x0s = np.zeros((N_CORES, P, TOT), np.float32)
    x1s = np.zeros((N_CORES, P, TOT), np.float32)
    dgs = np.ones((N_CORES, P, TOT), np.float32)
    core_masks = []
    for c in range(N_CORES):
        m = ce == c
        core_masks.append(m)
        x0s[c][pe[m], pos[m]] = x[row[m], 0]
        x1s[c][pe[m], pos[m]] = x[row[m], 1]
        dgs[c][pe[m], pos[m]] = deg[row[m]]

    # ---- node tables ----
    pgrid = np.arange(P)[:, None]
    lgrid = np.arange(NSL)[None, :]
    x0t = np.zeros((N_CORES, P, NSL), np.float32)
    x1t = np.zeros((N_CORES, P, NSL), np.float32)
    dgt = np.ones((N_CORES, P, NSL), np.float32)
    nodes_c = []
    valid_c = []
    for c in range(N_CORES):
        ranks = (lgrid * N_CORES + c) * P + pgrid          # [P, NSL]
        valid = ranks < N
        nodes = order[np.minimum(ranks, N - 1)]
        nodes_c.append(nodes); valid_c.append(valid)
        x0t[c] = np.where(valid, x[nodes, 0], 0.0)
        x1t[c] = np.where(valid, x[nodes, 1], 0.0)
        dgt[c] = np.where(valid, deg[nodes].astype(np.float32), 1.0)

    wb = np.zeros((P, 65), np.float32)
    wb[:, 0:16] = W1[0]; wb[:, 16:32] = W1[1]
    wb[:, 32:48] = b1
    wb[:, 48:64] = W2[:, 0]
    wb[:, 64] = b2[0]

    in_maps_A = [{"x0s": x0s[c], "x1s": x1s[c], "dgs": dgs[c],
                  "x0t": x0t[c], "x1t": x1t[c], "dgt": dgt[c], "wb": wb}
                 for c in range(N_CORES)]
    resA = run_bass_kernel_spmd(ncA, in_maps_A, core_ids=list(range(N_CORES)),
                                trace=TRACE)

    # ---- sigma table, host re-shard into slots ----
    sigma = np.zeros(N, np.float32)
    for c in range(N_CORES):
        v = valid_c[c]
        sigma[nodes_c[c][v]] = resA.results[c]["sOut"][v]

    sgs = np.zeros((N_CORES, P, TOT), np.float32)
    st = np.zeros((N_CORES, P, NSL), np.float32)
    for c in range(N_CORES):
        m = core_masks[c]
        sgs[c][pe[m], pos[m]] = sigma[row[m]]
        st[c] = np.where(valid_c[c], sigma[nodes_c[c]], 0.0)

    in_maps_B = [{"sgs": sgs[c], "dgtB": dgt[c], "stB": st[c], "wbB": wb}
                 for c in range(N_CORES)]
    resB = run_bass_kernel_spmd(ncB, in_maps_B, core_ids=list(range(N_CORES)),
                                trace=TRACE)

    out = np.zeros(N, np.float32)
    for c in range(N_CORES):
        v = valid_c[c]
        out[nodes_c[c][v]] = resB.results[c]["out"][v]

    kernel.last_exec_ns = (getattr(resA, "exec_time_ns", None) or 0) + \
                          (getattr(resB, "exec_time_ns", None) or 0)
    return out[:, None]
